# revision 5
# baseline (speedup 1.0000x reference)
"""Trainium2 Bass kernel for nn_CompressiveMemory_57750130262084.

The reference computes (B=8, S=4096, DK=DV=1024):
    sigma  = elu(query) + 1                                  [B,S,DK]
    memory = einsum('bkd,bsv->bkv', swap(sigma), value)      [B,DK,DV]
    z_norm = sum_s sigma                                     [B,DK]
    out    = einsum('bsd,bkv->bsv', sigma, memory)
           / einsum('bsd,bk->bs',  sigma, z_norm)[..., None]

Every einsum uses disjoint summed subscripts, so each factorises into
outer products of independent reductions:
    memory[b,k,v]    = z_norm[b,k] * VS[b,v]      with VS[b,v] = sum_s value[b,s,v]
    retrieved[b,s,v] = rs[b,s] * Z[b] * VS[b,v]   with rs = rowsum(sigma), Z = sum_k z_norm
    denom[b,s]       = rs[b,s] * Z[b]
    out[b,s,v]       = VS[b,v]                    (exactly; query cancels)

So the kernel is a column-sum of `value` over S, broadcast over S.
Sharding: data-parallel over batch, one NeuronCore per batch element.
Per-core work: read 16 MB, reduce 4096 rows -> 1 row, write 16 MB;
memory-bound at the ~358 GB/s per-NC HBM limit.

Schedule per core:
  - input as DMAs of descending size (chunks of 128 rows x 1024 cols);
    fp32 tensor_tensor adds (1x mode, ~1.23 us/chunk) chase the DMAs,
    so only the last small chunk's add sits on the critical tail
  - partition-reduce + broadcast in one step: ones[128,128]^T @ acc
    on the PE -> PSUM[128,1024] where every partition holds the colsum
  - copy PSUM->SBUF once; output DMAs use a step-0 (broadcast) source
    AP to fan the single [128,1024] tile out to all 4096 rows
"""

import numpy as np

B, S, D = 8, 4096, 1024
P = 128                 # SBUF partitions
N_CHUNK = S // P        # 32 row-chunks of 128 rows
IN_SIZES = [2, 4, 8, 8, 4, 2, 2, 1, 1]   # chunks per input DMA (sum = 32)
OUT_REP = 8             # row-chunks per output DMA -> 4 MB writes
N_OUT = N_CHUNK // OUT_REP

_CACHE: dict = {}


def _build_program():
    import concourse.mybir as mybir
    import concourse.tile as tile
    from concourse import bacc

    assert sum(IN_SIZES) == N_CHUNK
    f32 = mybir.dt.float32
    nc = bacc.Bacc("TRN2", target_bir_lowering=False, debug=False, num_devices=B)
    v = nc.declare_dram_parameter("value", [S, D], f32, isOutput=False)
    o = nc.declare_dram_parameter("out", [S, D], f32, isOutput=True)

    v_rows = v[:].rearrange("(c p) m -> c p m", p=P)       # [32][128][1024]
    o_re = o[:].rearrange("(i n p) m -> i p n m", i=N_OUT, n=OUT_REP, p=P)

    with tile.TileContext(nc) as tc:
        with (
            tc.tile_pool(name="in", bufs=1) as in_pool,
            tc.tile_pool(name="acc", bufs=1) as acc_pool,
            tc.tile_pool(name="ones", bufs=1) as ones_pool,
            tc.tile_pool(name="bcast", bufs=1) as bcast_pool,
            tc.tile_pool(name="psum", bufs=1, space="PSUM") as psum_pool,
        ):
            ones = ones_pool.tile([P, P], f32)
            nc.vector.memset(ones[:], 1.0)

            acc = acc_pool.tile([P, D], f32)
            chunk0 = 0
            n_adds = 0
            for ti, sz in enumerate(IN_SIZES):
                t = in_pool.tile([P, sz * D], f32, tag=f"in{ti}")
                # DRAM side: rows [chunk0*128, (chunk0+sz)*128)
                src = v_rows[chunk0 : chunk0 + sz].rearrange("n p m -> p n m")
                nc.sync.dma_start(t[:].rearrange("p (n m) -> p n m", n=sz), src)
                for n in range(sz):
                    sl = t[:, n * D : (n + 1) * D]
                    if n_adds == 0:
                        nc.vector.tensor_copy(acc[:], sl)
                    else:
                        nc.vector.tensor_add(acc[:], acc[:], sl)
                    n_adds += 1
                chunk0 += sz

            # Partition reduce + broadcast: psum[p, f] = sum_k acc[k, f] for all p
            ps = psum_pool.tile([P, D], f32)
            nc.tensor.matmul(ps[:, 0:512], ones[:], acc[:, 0:512], start=True, stop=True)
            nc.tensor.matmul(ps[:, 512:D], ones[:], acc[:, 512:D], start=True, stop=True)

            bc = bcast_pool.tile([P, D], f32)
            nc.vector.tensor_copy(bc[:], ps[:])

            src = bc[:].unsqueeze(1).to_broadcast((P, OUT_REP, D))
            for i in range(N_OUT):
                nc.sync.dma_start(o_re[i], src)

    nc.compile()
    return nc


def _get_program():
    if "nc" not in _CACHE:
        _CACHE["nc"] = _build_program()
    return _CACHE["nc"]


def kernel(query: np.ndarray, value: np.ndarray) -> np.ndarray:
    from concourse.bass_utils import run_bass_kernel_spmd

    del query  # output is exactly independent of query (see module docstring)
    value = np.ascontiguousarray(value, dtype=np.float32)
    assert value.shape == (B, S, D)

    nc = _get_program()
    in_maps = [{"value": value[b]} for b in range(B)]
    res = run_bass_kernel_spmd(nc, in_maps, list(range(B)))
    return np.stack([res.results[b]["out"] for b in range(B)], axis=0)


# revision 6
# speedup vs baseline: 1.0662x; 1.0662x over previous
"""Trainium2 Bass kernel for nn_CompressiveMemory_57750130262084.

The reference computes (B=8, S=4096, DK=DV=1024):
    sigma  = elu(query) + 1                                  [B,S,DK]
    memory = einsum('bkd,bsv->bkv', swap(sigma), value)      [B,DK,DV]
    z_norm = sum_s sigma                                     [B,DK]
    out    = einsum('bsd,bkv->bsv', sigma, memory)
           / einsum('bsd,bk->bs',  sigma, z_norm)[..., None]

Every einsum uses disjoint summed subscripts, so each factorises into
outer products of independent reductions:
    memory[b,k,v]    = z_norm[b,k] * VS[b,v]      with VS[b,v] = sum_s value[b,s,v]
    retrieved[b,s,v] = rs[b,s] * Z[b] * VS[b,v]   with rs = rowsum(sigma), Z = sum_k z_norm
    denom[b,s]       = rs[b,s] * Z[b]
    out[b,s,v]       = VS[b,v]                    (exactly; query cancels)

So the kernel is a column-sum of `value` over S, broadcast over S.
Sharding: data-parallel over batch, one NeuronCore per batch element.
Per-core work: read 16 MB, reduce 4096 rows -> 1 row, write 16 MB;
memory-bound at the ~358 GB/s per-NC HBM limit.

Schedule per core:
  - input as DMAs of descending size (chunks of 128 rows x 1024 cols);
    fp32 tensor_tensor adds (1x mode, ~1.23 us/chunk) chase the DMAs,
    so only the last small chunk's add sits on the critical tail
  - partition-reduce + broadcast in one step: ones[128,128]^T @ acc
    on the PE -> PSUM[128,1024] where every partition holds the colsum
  - copy PSUM->SBUF once; output DMAs use a step-0 (broadcast) source
    AP to fan the single [128,1024] tile out to all 4096 rows
"""

import numpy as np

B, S, D = 8, 4096, 1024
P = 128                 # SBUF partitions
N_CHUNK = S // P        # 32 row-chunks of 128 rows
IN_SIZES = [2, 4, 8, 8, 4, 2, 2, 1, 1]   # chunks per input DMA (sum = 32)
OUT_REP = 8             # row-chunks per output DMA -> 4 MB writes
N_OUT = N_CHUNK // OUT_REP

_CACHE: dict = {}


def _build_program():
    import concourse.mybir as mybir
    import concourse.tile as tile
    from concourse import bacc

    assert sum(IN_SIZES) == N_CHUNK
    f32 = mybir.dt.float32
    nc = bacc.Bacc("TRN2", target_bir_lowering=False, debug=False, num_devices=B)
    v = nc.declare_dram_parameter("value", [S, D], f32, isOutput=False)
    o = nc.declare_dram_parameter("out", [S, D], f32, isOutput=True)

    v_rows = v[:].rearrange("(c p) m -> c p m", p=P)       # [32][128][1024]
    o_re = o[:].rearrange("(i n p) m -> i p n m", i=N_OUT, n=OUT_REP, p=P)

    with tile.TileContext(nc) as tc:
        with (
            tc.tile_pool(name="in", bufs=1) as in_pool,
            tc.tile_pool(name="acc", bufs=1) as acc_pool,
            tc.tile_pool(name="ones", bufs=1) as ones_pool,
            tc.tile_pool(name="bcast", bufs=1) as bcast_pool,
            tc.tile_pool(name="psum", bufs=1, space="PSUM") as psum_pool,
        ):
            ones = ones_pool.tile([P, P], f32)
            nc.vector.memset(ones[:], 1.0)

            # Split the 32 chunk-reductions between DVE (tensor_add chain into
            # acc, fp32 capped at 1x mode) and the otherwise-idle PE (PSUM-
            # accumulating ones^T @ chunk, which also partition-reduces and
            # broadcasts for free). Every 3rd chunk goes to the PE.
            ps = psum_pool.tile([P, D], f32)
            acc = acc_pool.tile([P, D], f32)
            chunk0 = 0
            n_dve = 0
            n_pe = 0
            for ti, sz in enumerate(IN_SIZES):
                t = in_pool.tile([P, sz * D], f32, tag=f"in{ti}")
                # DRAM side: rows [chunk0*128, (chunk0+sz)*128)
                src = v_rows[chunk0 : chunk0 + sz].rearrange("n p m -> p n m")
                nc.sync.dma_start(t[:].rearrange("p (n m) -> p n m", n=sz), src)
                for n in range(sz):
                    sl = t[:, n * D : (n + 1) * D]
                    if (chunk0 + n) % 3 == 2:
                        for h in range(2):
                            nc.tensor.matmul(
                                ps[:, h * 512 : (h + 1) * 512],
                                ones[:],
                                sl[:, h * 512 : (h + 1) * 512],
                                start=(n_pe == 0),
                                stop=False,
                            )
                        n_pe += 1
                    elif n_dve == 0:
                        nc.vector.tensor_copy(acc[:], sl)
                        n_dve += 1
                    else:
                        nc.vector.tensor_add(acc[:], acc[:], sl)
                        n_dve += 1
                chunk0 += sz

            # Fold the DVE accumulator into PSUM; ends both accumulation groups.
            for h in range(2):
                nc.tensor.matmul(
                    ps[:, h * 512 : (h + 1) * 512],
                    ones[:],
                    acc[:, h * 512 : (h + 1) * 512],
                    start=False,
                    stop=True,
                )

            bc = bcast_pool.tile([P, D], f32)
            nc.vector.tensor_copy(bc[:], ps[:])

            src = bc[:].unsqueeze(1).to_broadcast((P, OUT_REP, D))
            for i in range(N_OUT):
                nc.sync.dma_start(o_re[i], src)

    nc.compile()
    return nc


def _get_program():
    if "nc" not in _CACHE:
        _CACHE["nc"] = _build_program()
    return _CACHE["nc"]


def kernel(query: np.ndarray, value: np.ndarray) -> np.ndarray:
    from concourse.bass_utils import run_bass_kernel_spmd

    del query  # output is exactly independent of query (see module docstring)
    value = np.ascontiguousarray(value, dtype=np.float32)
    assert value.shape == (B, S, D)

    nc = _get_program()
    in_maps = [{"value": value[b]} for b in range(B)]
    res = run_bass_kernel_spmd(nc, in_maps, list(range(B)))
    return np.stack([res.results[b]["out"] for b in range(B)], axis=0)
